# revision 15
# baseline (speedup 1.0000x reference)
"""Fused 6-layer MLP (AbsoluteNeuralNetwork) for 8 TRN2 NeuronCores.

Data-parallel over batch: each core gets B/8 = 4096 rows. Weights are
replicated. Activations live in SBUF in transposed [dim, batch] layout for
the whole 6-layer stack; weights stream from DRAM per layer. The cos-product
abs_out needs only aw[:, :, :3] (the reference math reduces the circulant
gate product to it), so only a [1024, 27]-repacked slice is shipped per abs
layer and the product is computed on-device.

Matmul precision per layer is configurable:
  - "f16x2": fp32-grade via 3-pass fp16 hi/lo split (wh@ah + wh@al + wl@ah),
             3 PE-cycles/row instead of fp32's 4.
  - "f16":   single-pass fp16 (used only where the network's chaotic
             amplification tolerates the 2^-11 injection).
  - "f32":   native fp32 matmul (4 cycles/row).
"""

import math
import os
from contextlib import ExitStack

import numpy as np

import concourse.bass as bass
import concourse.mybir as mybir
from concourse import bacc
from concourse.tile import TileContext
from concourse.bass_utils import run_bass_kernel_spmd

B = 32768
D = 1024
NCORES = 8
BLOC = B // NCORES  # 4096
KT = D // 128       # 8 k-tiles
MT = D // 128       # 8 dout-tiles

F32 = mybir.dt.float32
F16 = mybir.dt.float16

# layer -> (weight name, activation, bias source)
LAYERS = [
    ("w0", "relu", "b0"),
    ("w1", "relu", "b1"),
    ("cw1", "tanh", 0),   # tanh bias = cb1 + absvec(aw1)
    ("w2", "relu", "b2"),
    ("cw2", "tanh", 1),   # tanh bias = cb2 + absvec(aw2)
    ("w3", "relu", "b3"),
]
RELU_BIAS_COL = {0: 0, 1: 1, 3: 2, 5: 3}  # layer -> column group in packed relu biases
CB_COL = {2: 0, 4: 1}                     # tanh layer -> column group in packed cb

DEFAULT_CFG = os.environ.get("BASS_CFG", "f16x2,f16x2,f16x2,f16x2w,f16x2,f16")
_nc_cache = {}


def build(cfg: tuple[str, ...], stop_after: int | None = None, loop_n: int = 0):
    """Build + compile the SPMD Bass module for the given per-layer modes.

    loop_n > 0 wraps the layer stack in a hardware For_i loop executing it
    loop_n times — benchmarking only (iterations >1 read self-overwritten
    activations; matmul/act timing is data-independent)."""
    all_f32 = all(m == "f32" for m in cfg)
    assert all_f32 or all(m in ("f16x2", "f16x2w", "f16") for m in cfg), cfg
    chunk = 512   # one PSUM bank per matmul caps the moving dim at 512 fp32
    nch = BLOC // chunk

    nc = bacc.Bacc(None, target_bir_lowering=False)

    # ---- DRAM parameters -------------------------------------------------
    if all_f32:
        xf = nc.declare_dram_parameter("xf", [D, BLOC], F32, isOutput=False)
    else:
        xh = nc.declare_dram_parameter("xh", [D, BLOC], F16, isOutput=False)
        xl = nc.declare_dram_parameter("xl", [D, BLOC], F16, isOutput=False)
    wparams = []
    for li, (wname, _, _) in enumerate(LAYERS):
        if cfg[li] == "f32":
            wparams.append(
                (nc.declare_dram_parameter(f"{wname}f", [D, D], F32, isOutput=False),)
            )
        elif cfg[li] == "f16":
            wparams.append(
                (nc.declare_dram_parameter(f"{wname}h", [D, D], F16, isOutput=False),)
            )
        else:  # f16x2 / f16x2w
            wparams.append(
                (
                    nc.declare_dram_parameter(f"{wname}h", [D, D], F16, isOutput=False),
                    nc.declare_dram_parameter(f"{wname}l", [D, D], F16, isOutput=False),
                )
            )
    # packed per-partition vectors: relu biases [128, 4*8], cb [128, 2*8],
    # aw cos slices [128, 2*8*27]
    bpk = nc.declare_dram_parameter("bpk", [128, 4 * MT], F32, isOutput=False)
    cbk = nc.declare_dram_parameter("cbk", [128, 2 * MT], F32, isOutput=False)
    awc = nc.declare_dram_parameter("awc", [128, 2 * MT * 27], F32, isOutput=False)
    outT = nc.declare_dram_parameter("outT", [D, BLOC], F32, isOutput=True)

    hdt = F32 if all_f32 else F16

    with TileContext(nc) as tc:
        with ExitStack() as ctx:
            singles = ctx.enter_context(tc.tile_pool(name="singles", bufs=1))
            setup = ctx.enter_context(tc.tile_pool(name="setup", bufs=4))
            hpool = ctx.enter_context(tc.tile_pool(name="h", bufs=1))
            wpool = ctx.enter_context(tc.tile_pool(name="w", bufs=(14 if all_f32 else 24)))
            scratch = ctx.enter_context(tc.tile_pool(name="scr", bufs=4))
            psum = ctx.enter_context(tc.tile_pool(name="ps", bufs=8, space="PSUM"))

            # ---- persistent activation tiles (in-place across layers) ----
            hi = [[None] * nch for _ in range(KT)]
            lo = [[None] * nch for _ in range(KT)]
            for c in range(nch):
                for k in range(KT):
                    t = hpool.tile([128, chunk], hdt, name=f"hi_{k}_{c}", tag=f"hi_{k}_{c}")
                    src = xf if all_f32 else xh
                    nc.sync.dma_start(
                        out=t, in_=src[k * 128:(k + 1) * 128, c * chunk:(c + 1) * chunk]
                    )
                    hi[k][c] = t
                    if not all_f32:
                        t = hpool.tile([128, chunk], F16, name=f"lo_{k}_{c}", tag=f"lo_{k}_{c}")
                        nc.sync.dma_start(
                            out=t, in_=xl[k * 128:(k + 1) * 128, c * chunk:(c + 1) * chunk]
                        )
                        lo[k][c] = t

            # ---- bias / abs setup ----------------------------------------
            bpk_t = singles.tile([128, 4 * MT], F32)
            nc.sync.dma_start(out=bpk_t, in_=bpk[:, :])
            cbk_t = singles.tile([128, 2 * MT], F32)
            nc.sync.dma_start(out=cbk_t, in_=cbk[:, :])
            awc_t = singles.tile([128, 2 * MT * 27], F32)
            nc.sync.dma_start(out=awc_t, in_=awc[:, :])
            pio2 = singles.tile([128, 1], F32)
            nc.vector.memset(pio2, math.pi / 2)

            tanh_bias = [[None] * MT for _ in range(2)]
            for a in range(2):
                for m in range(MT):
                    off = (a * MT + m) * 27
                    s = setup.tile([128, 27], F32, name="abs_s", tag="abs_s")
                    # cos(x) = sin(x + pi/2)
                    nc.scalar.activation(
                        s, awc_t[:, off:off + 27],
                        mybir.ActivationFunctionType.Sin, bias=pio2, scale=1.0,
                    )
                    sq = setup.tile([128, 27], F32, name="abs_sq", tag="abs_sq")
                    nc.vector.tensor_mul(sq, s, s)
                    p = setup.tile([128, 1], F32, name="abs_p", tag="abs_p")
                    nc.vector.tensor_reduce(
                        out=p, in_=sq, op=mybir.AluOpType.mult, axis=mybir.AxisListType.X
                    )
                    pm = setup.tile([128, 1], F32, name="abs_pm", tag="abs_pm")
                    nc.scalar.mul(pm, p, 1.0 / D)
                    tb = singles.tile([128, 1], F32, name=f"tb_{a}_{m}")
                    col = a * MT + m
                    nc.vector.tensor_add(tb, pm, cbk_t[:, col:col + 1])
                    tanh_bias[a][m] = tb

            # ---- main layer loop -----------------------------------------
            from contextlib import nullcontext
            loop_ctx = tc.For_i(0, loop_n, 1) if loop_n else nullcontext()
            with loop_ctx:
                _emit_layers(nc, cfg, stop_after, chunk, nch, wparams,
                             hi, lo, wpool, scratch, psum, bpk_t, tanh_bias, outT)

    nc.compile()
    return nc, cfg, chunk


def _emit_layers(nc, cfg, stop_after, chunk, nch, wparams, hi, lo,
                 wpool, scratch, psum, bpk_t, tanh_bias, outT):
    if True:
        if True:
            for li, (wname, actfn, bsrc) in enumerate(LAYERS):
                mode = cfg[li]
                wdt = F32 if mode == "f32" else F16
                wh_t, wl_t = [], []
                for k in range(KT):
                    t = wpool.tile([128, D], wdt, name=f"wh{li}_{k}", tag="w")
                    nc.sync.dma_start(out=t, in_=wparams[li][0][k * 128:(k + 1) * 128, :])
                    wh_t.append(t)
                if mode in ("f16x2", "f16x2w"):
                    for k in range(KT):
                        t = wpool.tile([128, D], F16, name=f"wl{li}_{k}", tag="w")
                        nc.sync.dma_start(out=t, in_=wparams[li][1][k * 128:(k + 1) * 128, :])
                        wl_t.append(t)

                next_mode = cfg[li + 1] if li + 1 < 6 else None
                for c in range(nch):
                    # Phase 1: all MT matmul groups of this chunk into MT psum
                    # banks. Phase 2: activations (which overwrite hi/lo in
                    # place) only after every group has read the old values.
                    pss = []
                    for m in range(MT):
                        ps = psum.tile([128, chunk], F32, name="ps", tag="ps")
                        pss.append(ps)
                        msl = slice(m * 128, (m + 1) * 128)
                        if mode in ("f16x2", "f16x2w"):
                            passes = 3 if mode == "f16x2" else 2
                            n_mm = passes * KT
                            i = 0
                            for k in range(KT):
                                pass_list = (
                                    (wh_t[k][:, msl], hi[k][c]),
                                    (wh_t[k][:, msl], lo[k][c]),
                                    (wl_t[k][:, msl], hi[k][c]),
                                ) if mode == "f16x2" else (
                                    (wh_t[k][:, msl], hi[k][c]),
                                    (wl_t[k][:, msl], hi[k][c]),
                                )
                                for lhsT, rhs in pass_list:
                                    nc.tensor.matmul(
                                        ps, lhsT=lhsT, rhs=rhs,
                                        start=(i == 0), stop=(i == n_mm - 1),
                                    )
                                    i += 1
                        else:
                            for k in range(KT):
                                nc.tensor.matmul(
                                    ps, lhsT=wh_t[k][:, msl], rhs=hi[k][c],
                                    start=(k == 0), stop=(k == KT - 1),
                                )

                    for m in range(MT):
                        ps = pss[m]
                        msl = slice(m * 128, (m + 1) * 128)
                        if actfn == "relu":
                            func = mybir.ActivationFunctionType.Relu
                            bias = bpk_t[:, RELU_BIAS_COL[li] * MT + m:RELU_BIAS_COL[li] * MT + m + 1]
                        else:
                            func = mybir.ActivationFunctionType.Tanh
                            bias = tanh_bias[bsrc][m]

                        csl = slice(c * chunk, (c + 1) * chunk)
                        if (stop_after is not None and li == stop_after) or li == 5:
                            o = scratch.tile([128, chunk], F32, name="o", tag="o")
                            nc.scalar.activation(o, ps, func, bias=bias, scale=1.0)
                            nc.sync.dma_start(out=outT[msl, csl], in_=o)
                        elif next_mode == "f16x2":
                            s = scratch.tile([128, chunk], F32, name="o", tag="o")
                            nc.scalar.activation(s, ps, func, bias=bias, scale=1.0)
                            nc.vector.tensor_copy(hi[m][c], s)
                            nc.vector.tensor_sub(lo[m][c], s, hi[m][c])
                        else:
                            # next layer reads a single-precision operand
                            nc.scalar.activation(hi[m][c], ps, func, bias=bias, scale=1.0)
                if stop_after is not None and li == stop_after:
                    break


def _get_nc(cfg, stop_after=None, loop_n=0):
    key = (tuple(cfg), stop_after, loop_n)
    if key not in _nc_cache:
        _nc_cache[key] = build(key[0], stop_after, loop_n)
    return _nc_cache[key]


def _pack_biases(b0, b1, b2, b3):
    bpk = np.empty((128, 4 * MT), np.float32)
    for i, b in enumerate((b0, b1, b2, b3)):
        bpk[:, i * MT:(i + 1) * MT] = b.reshape(MT, 128).T
    return bpk


def _prep_inputs(inputs, cfg):
    """Host-side shard/split/pack. Returns per-core in_maps."""
    all_f32 = all(m == "f32" for m in cfg)
    ins = {k: np.ascontiguousarray(np.asarray(v, dtype=np.float32)) for k, v in inputs.items()}

    shared = {}
    for li, (wname, _, _) in enumerate(LAYERS):
        w = ins[wname]
        if cfg[li] == "f32":
            shared[f"{wname}f"] = w
        elif cfg[li] == "f16":
            shared[f"{wname}h"] = w.astype(np.float16)
        else:
            wh = w.astype(np.float16)
            shared[f"{wname}h"] = wh
            shared[f"{wname}l"] = (w - wh.astype(np.float32)).astype(np.float16)

    shared["bpk"] = _pack_biases(ins["b0"], ins["b1"], ins["b2"], ins["b3"])
    cbk = np.empty((128, 2 * MT), np.float32)
    cbk[:, :MT] = ins["cb1"].reshape(MT, 128).T
    cbk[:, MT:] = ins["cb2"].reshape(MT, 128).T
    shared["cbk"] = cbk
    awc = np.empty((128, 2 * MT * 27), np.float32)
    for a, aw in enumerate((ins["aw1"], ins["aw2"])):
        sl = aw[:, :, :3].transpose(1, 0, 2).reshape(D, 27)  # [j, 27]
        awc[:, a * MT * 27:(a + 1) * MT * 27] = (
            sl.reshape(MT, 128, 27).transpose(1, 0, 2).reshape(128, MT * 27)
        )
    shared["awc"] = awc

    in_maps = []
    x = ins["x"]
    for core in range(NCORES):
        xT = np.ascontiguousarray(x[core * BLOC:(core + 1) * BLOC, :].T)
        m = dict(shared)
        if all_f32:
            m["xf"] = xT
        else:
            xh = xT.astype(np.float16)
            m["xh"] = xh
            m["xl"] = (xT - xh.astype(np.float32)).astype(np.float16)
        in_maps.append(m)
    return in_maps


def run(inputs, cfg=None, stop_after=None, **spmd_kwargs):
    cfg = tuple((cfg or DEFAULT_CFG).split(",")) if isinstance(cfg or DEFAULT_CFG, str) else tuple(cfg)
    nc, cfg, _ = _get_nc(cfg, stop_after)
    in_maps = _prep_inputs(inputs, cfg)
    res = None
    for attempt in range(3):
        try:
            res = run_bass_kernel_spmd(nc, in_maps, list(range(NCORES)), **spmd_kwargs)
            break
        except Exception:
            # transient device wedges (NRT_EXEC_UNIT_UNRECOVERABLE) clear on retry
            if attempt == 2:
                raise
    out = np.empty((B, D), np.float32)
    for core in range(NCORES):
        out[core * BLOC:(core + 1) * BLOC, :] = res.results[core]["outT"].T
    return out, res


def kernel(**inputs) -> np.ndarray:
    out, _ = run(inputs)
    return out
